# revision 12
# baseline (speedup 1.0000x reference)
"""Trainium2 Bass kernel for nn_EnhanceDiversityFeatureExtracition.

Computes  loss = mean((x-y)^2) + ALPHA * diversity_reg(conv_w)
where diversity_reg builds a 64x64 Gram matrix of the F=64 slices
conv_w[:, :, i, :] (each flattened to a 786432-vector), normalizes it to
cosine similarities, and sums the entries with tau < sim <= 1 off the
diagonal.

Distribution (8 NeuronCores, SPMD):
  - x_batch / y_batch sharded on batch dim: 256 rows per core.
  - conv_w viewed as A = conv_w.reshape(262144, 192)  (row m = (o,c),
    col = f*3+k).  gram[i,j] = sum_m sum_k A[m,3i+k]*A[m,3j+k], so A is
    sharded along the 262144-row reduction axis: 32768 rows per core.
  - Each core returns its partial C = A^T A (192x192) and per-partition
    partial sums of (x-y)^2; the host sums the partials, extracts the
    k-diagonals gram[i,j] = sum_k C[3i+k,3j+k], and applies the tiny
    64x64 masked-similarity epilogue.

On-core dataflow:
  - A shard is streamed in 32 blocks of 1024 rows laid out as
    [128 partitions x 1536 floats] (per-partition contiguous 6KB DMA).
    Each block yields 8*2 = 16 matmuls (contraction 128, strided
    256-wide fp32r moving operands) accumulating into two PSUM tiles
    (C rows 0-127 and 128-191) across the whole shard.
  - DMA issue is split across both HWDGE queues (sync + scalar
    engines) so descriptor generation is not serialized on one queue.
  - MSE: 4 chunks of [128 x 2048] per operand; DVE computes d = x-y,
    ACT computes Square(d) with a per-partition accumulate.
"""

import numpy as np

import concourse.bass as bass
import concourse.mybir as mybir
from concourse import bacc, tile
from concourse.bass_utils import run_bass_kernel_spmd

N_CORES = 8
B, D = 2048, 4096            # x_batch / y_batch
M, G = 262144, 192           # conv_w as (M, G); G = F*KW
F, KW = 64, 3
ROWS = B // N_CORES          # 256 batch rows per core
MC = M // N_CORES            # 32768 reduction rows per core
TPB = 8                      # 128-row tiles per DMA block
BLK = 128 * TPB              # 1024 rows per block
NBLK = MC // BLK             # 32
NCH = 8                      # MSE chunks per core
CHW = (ROWS * D) // (128 * NCH)  # 1024 floats per partition per chunk
XYB = 4                      # one MSE chunk rides every XYB-th A block

ALPHA = 0.0005
TAU = 0.2

_prog = None


def _build() -> bass.Bass:
    nc = bacc.Bacc(None, target_bir_lowering=False)
    f32 = mybir.dt.float32
    f32r = mybir.dt.float32r

    xs = nc.dram_tensor("xs", [ROWS, D], f32, kind="ExternalInput")
    ys = nc.dram_tensor("ys", [ROWS, D], f32, kind="ExternalInput")
    aw = nc.dram_tensor("aw", [MC, G], f32r, kind="ExternalInput")
    # packed output: cols 0:192 = C rows 0-127; cols 192:384 (rows 0-63)
    # = C rows 128-191; cols 384:384+NCH = per-partition MSE partials
    out_d = nc.dram_tensor("out_d", [128, 2 * G + NCH], f32, kind="ExternalOutput")

    # moving operand width for the fp32r full-rate mode (must be >= 256)
    RW = 256
    PAD = RW - G  # 64 junk columns beyond each 192-col tile (never read back)

    with tile.TileContext(nc) as tc:
        with (
            tc.tile_pool(name="apool", bufs=12) as apool,
            tc.tile_pool(name="xpool", bufs=NCH) as xpool,
            tc.tile_pool(name="ypool", bufs=NCH) as ypool,
            tc.tile_pool(name="dpool", bufs=4) as dpool,
            tc.tile_pool(name="qpool", bufs=4) as qpool,
            tc.tile_pool(name="opool", bufs=1) as opool,
            tc.tile_pool(name="psum", bufs=1, space=bass.MemorySpace.PSUM) as psum,
        ):
            # C = A^T A accumulator, rows 0-127 and 128-191
            cps1 = psum.tile([128, RW], f32, tag="cps1")
            cps2 = psum.tile([F, RW], f32, tag="cps2")
            osb = opool.tile([128, 2 * G + NCH], f32, tag="osb")

            # per-partition contiguous views
            awv = aw[:].rearrange("(b p t) g -> b p (t g)", p=128, t=TPB)
            xv = xs[:].rearrange("(p t) d -> p (t d)", p=128)
            yv = ys[:].rearrange("(p t) d -> p (t d)", p=128)

            n_t = NBLK * TPB
            ti = 0
            for b in range(NBLK):
                # A block; tile is padded so the last sub-tile's 256-wide
                # moving operand stays in bounds.  The pad is never
                # initialized: its products only reach PSUM columns
                # 192-255, which are never read back.
                at = apool.tile([128, TPB * G + PAD], f32r)
                eng = nc.sync if (b % 2 == 0) else nc.scalar
                eng.dma_start(at[:, :TPB * G], awv[b])
                for t in range(TPB):
                    rhs = at[:, t * G:t * G + RW]
                    w1 = at[:, t * G:t * G + 128]
                    w2 = at[:, t * G + 128:t * G + G]
                    nc.tensor.matmul(
                        cps1[:], w1, rhs,
                        start=(ti == 0), stop=(ti == n_t - 1),
                    )
                    nc.tensor.matmul(
                        cps2[:], w2, rhs,
                        start=(ti == 0), stop=(ti == n_t - 1),
                    )
                    ti += 1

                # interleave one MSE chunk every XYB-th A block (last chunk
                # rides after the final A block): the x/y DMA time is the
                # slack that lets the tensor engine keep pace with the aw
                # stream, and the final chunk hides the matmul drain.
                if (b + 1) % XYB == 0:
                    ch = (b + 1) // XYB - 1
                    # the final chunk goes entirely through the scalar
                    # queue, FIFO-after the last aw block, so the matmul
                    # drain hides under the x/y stream
                    xeng = nc.scalar if ch == NCH - 1 else nc.sync
                    xt = xpool.tile([128, CHW], f32)
                    xeng.dma_start(xt[:], xv[:, ch * CHW:(ch + 1) * CHW])
                    yt = ypool.tile([128, CHW], f32)
                    nc.scalar.dma_start(yt[:], yv[:, ch * CHW:(ch + 1) * CHW])
                    dtile = dpool.tile([128, CHW], f32)
                    nc.vector.tensor_sub(dtile[:], xt[:], yt[:])
                    qtile = qpool.tile([128, CHW], f32)
                    nc.scalar.activation(
                        qtile[:], dtile[:],
                        mybir.ActivationFunctionType.Square,
                        accum_out=osb[:, 2 * G + ch:2 * G + ch + 1],
                    )

            # ---- pack C partials next to the MSE partials, single DMA out;
            # k-diagonal extraction happens on host
            nc.vector.tensor_copy(osb[:, 0:G], cps1[:, :G])
            nc.vector.tensor_copy(osb[0:F, G:2 * G], cps2[:, :G])
            nc.sync.dma_start(out_d[:], osb[:])

    nc.finalize()
    return nc


def _get_prog() -> bass.Bass:
    global _prog
    if _prog is None:
        _prog = _build()
    return _prog


def _epilogue(C: np.ndarray, sse: float) -> np.ndarray:
    Cr = C.reshape(F, KW, F, KW)
    gram = Cr[:, 0, :, 0] + Cr[:, 1, :, 1] + Cr[:, 2, :, 2]
    norms = np.sqrt(np.diag(gram))
    sim = gram / np.outer(norms, norms)
    mask = (sim > TAU) & (sim <= 1.0) & (~np.eye(F, dtype=bool))
    reg = sim[mask].sum()
    loss = sse / float(B * D) + ALPHA * reg
    return np.asarray(np.float32(loss))


def kernel(x_batch: np.ndarray, y_batch: np.ndarray, conv_w: np.ndarray) -> np.ndarray:
    nc = _get_prog()
    A = np.ascontiguousarray(conv_w.reshape(M, G))
    in_maps = []
    for c in range(N_CORES):
        in_maps.append({
            "xs": np.ascontiguousarray(x_batch[c * ROWS:(c + 1) * ROWS]),
            "ys": np.ascontiguousarray(y_batch[c * ROWS:(c + 1) * ROWS]),
            "aw": np.ascontiguousarray(A[c * MC:(c + 1) * MC]),
        })
    res = run_bass_kernel_spmd(nc, in_maps, core_ids=list(range(N_CORES))).results
    C = np.zeros((G, G), np.float64)
    sse = 0.0
    for r in res:
        o = r["out_d"].astype(np.float64)
        C[0:128] += o[:, 0:G]
        C[128:G] += o[0:F, G:2 * G]
        sse += float(o[:, 2 * G:].sum())
    return _epilogue(C, sse)


# revision 14
# speedup vs baseline: 1.1088x; 1.1088x over previous
"""Trainium2 Bass kernel for nn_EnhanceDiversityFeatureExtracition.

Computes  loss = mean((x-y)^2) + ALPHA * diversity_reg(conv_w)
where diversity_reg builds a 64x64 Gram matrix of the F=64 slices
conv_w[:, :, i, :] (each flattened to a 786432-vector), normalizes it to
cosine similarities, and sums the entries with tau < sim <= 1 off the
diagonal.

Distribution (8 NeuronCores, SPMD):
  - x_batch / y_batch sharded on batch dim: 256 rows per core.
  - conv_w viewed as A = conv_w.reshape(262144, 192)  (row m = (o,c),
    col = f*3+k).  gram[i,j] = sum_m sum_k A[m,3i+k]*A[m,3j+k], so A is
    sharded along the 262144-row reduction axis: 32768 rows per core.
  - Each core returns its partial C = A^T A (192x192) and per-partition
    partial sums of (x-y)^2; the host sums the partials, extracts the
    k-diagonals gram[i,j] = sum_k C[3i+k,3j+k], and applies the tiny
    64x64 masked-similarity epilogue.

On-core dataflow:
  - A shard is streamed in 32 blocks of 1024 rows laid out as
    [128 partitions x 1536 floats] (per-partition contiguous 6KB DMA).
    Each block yields 8*2 = 16 matmuls (contraction 128, strided
    256-wide fp32r moving operands) accumulating into two PSUM tiles
    (C rows 0-127 and 128-191) across the whole shard.
  - DMA issue is split across both HWDGE queues (sync + scalar
    engines) so descriptor generation is not serialized on one queue.
  - MSE: 4 chunks of [128 x 2048] per operand; DVE computes d = x-y,
    ACT computes Square(d) with a per-partition accumulate.
"""

import numpy as np

import concourse.bass as bass
import concourse.mybir as mybir
from concourse import bacc, tile
from concourse.bass_utils import run_bass_kernel_spmd

N_CORES = 8
B, D = 2048, 4096            # x_batch / y_batch
M, G = 262144, 192           # conv_w as (M, G); G = F*KW
F, KW = 64, 3
ROWS = B // N_CORES          # 256 batch rows per core
MC = M // N_CORES            # 32768 reduction rows per core
TPB = 8                      # 128-row tiles per DMA block
BLK = 128 * TPB              # 1024 rows per block
NBLK = MC // BLK             # 32
NCH = 8                      # MSE chunks per core
CHW = (ROWS * D) // (128 * NCH)  # 1024 floats per partition per chunk
XYB = 4                      # one MSE chunk rides every XYB-th A block

ALPHA = 0.0005
TAU = 0.2

_prog = None


def _build() -> bass.Bass:
    nc = bacc.Bacc(None, target_bir_lowering=False)
    f32 = mybir.dt.float32
    f32r = mybir.dt.float32r

    xs = nc.dram_tensor("xs", [ROWS, D], f32, kind="ExternalInput")
    ys = nc.dram_tensor("ys", [ROWS, D], f32, kind="ExternalInput")
    aw = nc.dram_tensor("aw", [MC, G], f32r, kind="ExternalInput")
    # packed output: cols 0:192 = C rows 0-127; cols 192:384 (rows 0-63)
    # = C rows 128-191; cols 384:384+NCH = per-partition MSE partials
    out_d = nc.dram_tensor("out_d", [128, 2 * G + NCH], f32, kind="ExternalOutput")

    # moving operand width for the fp32r full-rate mode (must be >= 256)
    RW = 256
    PAD = RW - G  # 64 junk columns beyond each 192-col tile (never read back)

    with tile.TileContext(nc) as tc:
        with (
            tc.tile_pool(name="apool", bufs=15) as apool,
            tc.tile_pool(name="xpool", bufs=NCH) as xpool,
            tc.tile_pool(name="ypool", bufs=NCH) as ypool,
            tc.tile_pool(name="dpool", bufs=2) as dpool,
            tc.tile_pool(name="qpool", bufs=2) as qpool,
            tc.tile_pool(name="opool", bufs=1) as opool,
            tc.tile_pool(name="psum", bufs=1, space=bass.MemorySpace.PSUM) as psum,
        ):
            # C = A^T A accumulator, rows 0-127 and 128-191
            cps1 = psum.tile([128, RW], f32, tag="cps1")
            cps2 = psum.tile([F, RW], f32, tag="cps2")
            osb = opool.tile([128, 2 * G + NCH], f32, tag="osb")

            # per-partition contiguous views
            awv = aw[:].rearrange("(b p t) g -> b p (t g)", p=128, t=TPB)
            xv = xs[:].rearrange("(p t) d -> p (t d)", p=128)
            yv = ys[:].rearrange("(p t) d -> p (t d)", p=128)

            n_t = NBLK * TPB
            ti = 0
            for b in range(NBLK):
                # A block; tile is padded so the last sub-tile's 256-wide
                # moving operand stays in bounds.  The pad is never
                # initialized: its products only reach PSUM columns
                # 192-255, which are never read back.
                at = apool.tile([128, TPB * G + PAD], f32r)
                eng = nc.sync if (b % 2 == 0) else nc.scalar
                eng.dma_start(at[:, :TPB * G], awv[b])
                for t in range(TPB):
                    rhs = at[:, t * G:t * G + RW]
                    w1 = at[:, t * G:t * G + 128]
                    w2 = at[:, t * G + 128:t * G + G]
                    nc.tensor.matmul(
                        cps1[:], w1, rhs,
                        start=(ti == 0), stop=(ti == n_t - 1),
                    )
                    nc.tensor.matmul(
                        cps2[:], w2, rhs,
                        start=(ti == 0), stop=(ti == n_t - 1),
                    )
                    ti += 1

                # interleave one MSE chunk every XYB-th A block (last chunk
                # rides after the final A block): the x/y DMA time is the
                # slack that lets the tensor engine keep pace with the aw
                # stream, and the final chunk hides the matmul drain.
                if (b + 1) % XYB == 0:
                    ch = (b + 1) // XYB - 1
                    xt = xpool.tile([128, CHW], f32)
                    nc.sync.dma_start(xt[:], xv[:, ch * CHW:(ch + 1) * CHW])
                    yt = ypool.tile([128, CHW], f32)
                    nc.scalar.dma_start(yt[:], yv[:, ch * CHW:(ch + 1) * CHW])
                    dtile = dpool.tile([128, CHW], f32)
                    nc.vector.tensor_sub(dtile[:], xt[:], yt[:])
                    qtile = qpool.tile([128, CHW], f32)
                    nc.scalar.activation(
                        qtile[:], dtile[:],
                        mybir.ActivationFunctionType.Square,
                        accum_out=osb[:, 2 * G + ch:2 * G + ch + 1],
                    )

            # ---- pack C partials next to the MSE partials, single DMA out;
            # k-diagonal extraction happens on host
            nc.vector.tensor_copy(osb[:, 0:G], cps1[:, :G])
            nc.vector.tensor_copy(osb[0:F, G:2 * G], cps2[:, :G])
            nc.sync.dma_start(out_d[:], osb[:])

    nc.finalize()
    return nc


def _get_prog() -> bass.Bass:
    global _prog
    if _prog is None:
        _prog = _build()
    return _prog


def _epilogue(C: np.ndarray, sse: float) -> np.ndarray:
    Cr = C.reshape(F, KW, F, KW)
    gram = Cr[:, 0, :, 0] + Cr[:, 1, :, 1] + Cr[:, 2, :, 2]
    norms = np.sqrt(np.diag(gram))
    sim = gram / np.outer(norms, norms)
    mask = (sim > TAU) & (sim <= 1.0) & (~np.eye(F, dtype=bool))
    reg = sim[mask].sum()
    loss = sse / float(B * D) + ALPHA * reg
    return np.asarray(np.float32(loss))


def kernel(x_batch: np.ndarray, y_batch: np.ndarray, conv_w: np.ndarray) -> np.ndarray:
    nc = _get_prog()
    A = np.ascontiguousarray(conv_w.reshape(M, G))
    in_maps = []
    for c in range(N_CORES):
        in_maps.append({
            "xs": np.ascontiguousarray(x_batch[c * ROWS:(c + 1) * ROWS]),
            "ys": np.ascontiguousarray(y_batch[c * ROWS:(c + 1) * ROWS]),
            "aw": np.ascontiguousarray(A[c * MC:(c + 1) * MC]),
        })
    res = run_bass_kernel_spmd(nc, in_maps, core_ids=list(range(N_CORES))).results
    C = np.zeros((G, G), np.float64)
    sse = 0.0
    for r in res:
        o = r["out_d"].astype(np.float64)
        C[0:128] += o[:, 0:G]
        C[128:G] += o[0:F, G:2 * G]
        sse += float(o[:, 2 * G:].sum())
    return _epilogue(C, sse)


# revision 15
# speedup vs baseline: 1.1316x; 1.0205x over previous
"""Trainium2 Bass kernel for nn_EnhanceDiversityFeatureExtracition.

Computes  loss = mean((x-y)^2) + ALPHA * diversity_reg(conv_w)
where diversity_reg builds a 64x64 Gram matrix of the F=64 slices
conv_w[:, :, i, :] (each flattened to a 786432-vector), normalizes it to
cosine similarities, and sums the entries with tau < sim <= 1 off the
diagonal.

Distribution (8 NeuronCores, SPMD):
  - x_batch / y_batch sharded on batch dim: 256 rows per core.
  - conv_w viewed as A = conv_w.reshape(262144, 192)  (row m = (o,c),
    col = f*3+k).  gram[i,j] = sum_m sum_k A[m,3i+k]*A[m,3j+k], so A is
    sharded along the 262144-row reduction axis: 32768 rows per core.
  - Each core returns its partial C = A^T A (192x192) and per-partition
    partial sums of (x-y)^2; the host sums the partials, extracts the
    k-diagonals gram[i,j] = sum_k C[3i+k,3j+k], and applies the tiny
    64x64 masked-similarity epilogue.

On-core dataflow:
  - A shard is streamed in 32 blocks of 1024 rows laid out as
    [128 partitions x 1536 floats] (per-partition contiguous 6KB DMA).
    Each block yields 8*2 = 16 matmuls (contraction 128, strided
    256-wide fp32r moving operands) accumulating into two PSUM tiles
    (C rows 0-127 and 128-191) across the whole shard.
  - DMA issue is split across both HWDGE queues (sync + scalar
    engines) so descriptor generation is not serialized on one queue.
  - MSE: 4 chunks of [128 x 2048] per operand; DVE computes d = x-y,
    ACT computes Square(d) with a per-partition accumulate.
"""

import numpy as np

import concourse.bass as bass
import concourse.mybir as mybir
from concourse import bacc, tile
from concourse.bass_utils import run_bass_kernel_spmd

N_CORES = 8
B, D = 2048, 4096            # x_batch / y_batch
M, G = 262144, 192           # conv_w as (M, G); G = F*KW
F, KW = 64, 3
ROWS = B // N_CORES          # 256 batch rows per core
MC = M // N_CORES            # 32768 reduction rows per core
TPB = 8                      # 128-row tiles per DMA block
BLK = 128 * TPB              # 1024 rows per block
NBLK = MC // BLK             # 32
NCH = 8                      # MSE chunks per core
CHW = (ROWS * D) // (128 * NCH)  # 1024 floats per partition per chunk
XYB = 4                      # one MSE chunk rides every XYB-th A block

ALPHA = 0.0005
TAU = 0.2

_prog = None


def _build() -> bass.Bass:
    nc = bacc.Bacc(None, target_bir_lowering=False)
    f32 = mybir.dt.float32
    f32r = mybir.dt.float32r

    xs = nc.dram_tensor("xs", [ROWS, D], f32, kind="ExternalInput")
    ys = nc.dram_tensor("ys", [ROWS, D], f32, kind="ExternalInput")
    aw = nc.dram_tensor("aw", [MC, G], f32r, kind="ExternalInput")
    # packed output: cols 0:192 = C rows 0-127; cols 192:384 (rows 0-63)
    # = C rows 128-191; cols 384:384+NCH = per-partition MSE partials
    out_d = nc.dram_tensor("out_d", [128, 2 * G + NCH], f32, kind="ExternalOutput")

    # moving operand width for the fp32r full-rate mode (must be >= 256)
    RW = 256
    PAD = RW - G  # 64 junk columns beyond each 192-col tile (never read back)

    with tile.TileContext(nc) as tc:
        with (
            tc.tile_pool(name="apool", bufs=15) as apool,
            tc.tile_pool(name="xpool", bufs=NCH) as xpool,
            tc.tile_pool(name="ypool", bufs=NCH) as ypool,
            tc.tile_pool(name="dpool", bufs=2) as dpool,
            tc.tile_pool(name="qpool", bufs=2) as qpool,
            tc.tile_pool(name="opool", bufs=1) as opool,
            tc.tile_pool(name="psum", bufs=1, space=bass.MemorySpace.PSUM) as psum,
        ):
            # C = A^T A accumulator, rows 0-127 and 128-191
            cps1 = psum.tile([128, RW], f32, tag="cps1")
            cps2 = psum.tile([F, RW], f32, tag="cps2")
            osb = opool.tile([128, 2 * G + NCH], f32, tag="osb")

            # per-partition contiguous views
            awv = aw[:].rearrange("(b p t) g -> b p (t g)", p=128, t=TPB)
            xv = xs[:].rearrange("(p t) d -> p (t d)", p=128)
            yv = ys[:].rearrange("(p t) d -> p (t d)", p=128)

            n_t = NBLK * TPB
            ti = 0
            for b in range(NBLK):
                # A block; tile is padded so the last sub-tile's 256-wide
                # moving operand stays in bounds.  The pad is never
                # initialized: its products only reach PSUM columns
                # 192-255, which are never read back.
                at = apool.tile([128, TPB * G + PAD], f32r)
                eng = nc.sync if (b % 2 == 0) else nc.scalar
                eng.dma_start(at[:, :TPB * G], awv[b])
                for t in range(TPB):
                    rhs = at[:, t * G:t * G + RW]
                    w1 = at[:, t * G:t * G + 128]
                    w2 = at[:, t * G + 128:t * G + G]
                    nc.tensor.matmul(
                        cps1[:], w1, rhs,
                        start=(ti == 0), stop=(ti == n_t - 1),
                    )
                    nc.tensor.matmul(
                        cps2[:], w2, rhs,
                        start=(ti == 0), stop=(ti == n_t - 1),
                    )
                    ti += 1

                # interleave one MSE chunk every XYB-th A block (last chunk
                # rides after the final A block): the x/y DMA time is the
                # slack that lets the tensor engine keep pace with the aw
                # stream, and the final chunk hides the matmul drain.
                if b % XYB == 1:
                    ch = b // XYB
                    xt = xpool.tile([128, CHW], f32)
                    nc.sync.dma_start(xt[:], xv[:, ch * CHW:(ch + 1) * CHW])
                    yt = ypool.tile([128, CHW], f32)
                    nc.scalar.dma_start(yt[:], yv[:, ch * CHW:(ch + 1) * CHW])
                    dtile = dpool.tile([128, CHW], f32)
                    nc.vector.tensor_sub(dtile[:], xt[:], yt[:])
                    qtile = qpool.tile([128, CHW], f32)
                    nc.scalar.activation(
                        qtile[:], dtile[:],
                        mybir.ActivationFunctionType.Square,
                        accum_out=osb[:, 2 * G + ch:2 * G + ch + 1],
                    )

            # ---- pack C partials next to the MSE partials, single DMA out;
            # k-diagonal extraction happens on host
            nc.vector.tensor_copy(osb[:, 0:G], cps1[:, :G])
            nc.vector.tensor_copy(osb[0:F, G:2 * G], cps2[:, :G])
            nc.sync.dma_start(out_d[:], osb[:])

    nc.finalize()
    return nc


def _get_prog() -> bass.Bass:
    global _prog
    if _prog is None:
        _prog = _build()
    return _prog


def _epilogue(C: np.ndarray, sse: float) -> np.ndarray:
    Cr = C.reshape(F, KW, F, KW)
    gram = Cr[:, 0, :, 0] + Cr[:, 1, :, 1] + Cr[:, 2, :, 2]
    norms = np.sqrt(np.diag(gram))
    sim = gram / np.outer(norms, norms)
    mask = (sim > TAU) & (sim <= 1.0) & (~np.eye(F, dtype=bool))
    reg = sim[mask].sum()
    loss = sse / float(B * D) + ALPHA * reg
    return np.asarray(np.float32(loss))


def kernel(x_batch: np.ndarray, y_batch: np.ndarray, conv_w: np.ndarray) -> np.ndarray:
    nc = _get_prog()
    A = np.ascontiguousarray(conv_w.reshape(M, G))
    in_maps = []
    for c in range(N_CORES):
        in_maps.append({
            "xs": np.ascontiguousarray(x_batch[c * ROWS:(c + 1) * ROWS]),
            "ys": np.ascontiguousarray(y_batch[c * ROWS:(c + 1) * ROWS]),
            "aw": np.ascontiguousarray(A[c * MC:(c + 1) * MC]),
        })
    res = run_bass_kernel_spmd(nc, in_maps, core_ids=list(range(N_CORES))).results
    C = np.zeros((G, G), np.float64)
    sse = 0.0
    for r in res:
        o = r["out_d"].astype(np.float64)
        C[0:128] += o[:, 0:G]
        C[128:G] += o[0:F, G:2 * G]
        sse += float(o[:, 2 * G:].sum())
    return _epilogue(C, sse)


# revision 18
# speedup vs baseline: 1.1404x; 1.0078x over previous
"""Trainium2 Bass kernel for nn_EnhanceDiversityFeatureExtracition.

Computes  loss = mean((x-y)^2) + ALPHA * diversity_reg(conv_w)
where diversity_reg builds a 64x64 Gram matrix of the F=64 slices
conv_w[:, :, i, :] (each flattened to a 786432-vector), normalizes it to
cosine similarities, and sums the entries with tau < sim <= 1 off the
diagonal.

Distribution (8 NeuronCores, SPMD):
  - x_batch / y_batch sharded on batch dim: 256 rows per core.
  - conv_w viewed as A = conv_w.reshape(262144, 192)  (row m = (o,c),
    col = f*3+k).  gram[i,j] = sum_m sum_k A[m,3i+k]*A[m,3j+k], so A is
    sharded along the 262144-row reduction axis: 32768 rows per core.
  - Each core returns its partial C = A^T A (192x192) and per-partition
    partial sums of (x-y)^2; the host sums the partials, extracts the
    k-diagonals gram[i,j] = sum_k C[3i+k,3j+k], and applies the tiny
    64x64 masked-similarity epilogue.

On-core dataflow:
  - A shard is streamed in 32 blocks of 1024 rows laid out as
    [128 partitions x 1536 floats] (per-partition contiguous 6KB DMA).
    Each block yields 8*2 = 16 matmuls (contraction 128, strided
    256-wide fp32r moving operands) accumulating into two PSUM tiles
    (C rows 0-127 and 128-191) across the whole shard.
  - DMA issue is split across both HWDGE queues (sync + scalar
    engines) so descriptor generation is not serialized on one queue.
  - MSE: 4 chunks of [128 x 2048] per operand; DVE computes d = x-y,
    ACT computes Square(d) with a per-partition accumulate.
"""

import numpy as np

import concourse.bass as bass
import concourse.mybir as mybir
from concourse import bacc, tile
from concourse.bass_utils import run_bass_kernel_spmd

N_CORES = 8
B, D = 2048, 4096            # x_batch / y_batch
M, G = 262144, 192           # conv_w as (M, G); G = F*KW
F, KW = 64, 3
ROWS = B // N_CORES          # 256 batch rows per core
MC = M // N_CORES            # 32768 reduction rows per core
TPB = 8                      # 128-row tiles per DMA block
BLK = 128 * TPB              # 1024 rows per block
NBLK = MC // BLK             # 32
NCH = 8                      # MSE chunks per core
CHW = (ROWS * D) // (128 * NCH)  # 1024 floats per partition per chunk
XYB = 4                      # one MSE chunk rides every XYB-th A block

ALPHA = 0.0005
TAU = 0.2

_prog = None


def _build() -> bass.Bass:
    nc = bacc.Bacc(None, target_bir_lowering=False)
    f32 = mybir.dt.float32
    f32r = mybir.dt.float32r

    xs = nc.dram_tensor("xs", [ROWS, D], f32, kind="ExternalInput")
    ys = nc.dram_tensor("ys", [ROWS, D], f32, kind="ExternalInput")
    aw = nc.dram_tensor("aw", [MC, G], f32r, kind="ExternalInput")
    # packed output: cols 0:192 = C rows 0-127; cols 192:384 (rows 0-63)
    # = C rows 128-191; cols 384:384+NCH = per-partition MSE partials
    out_d = nc.dram_tensor("out_d", [128, 2 * G + NCH], f32, kind="ExternalOutput")

    # moving operand width for the fp32r full-rate mode (must be >= 256)
    RW = 256
    PAD = RW - G  # 64 junk columns beyond each 192-col tile (never read back)

    with tile.TileContext(nc) as tc:
        with (
            tc.tile_pool(name="apool", bufs=15) as apool,
            tc.tile_pool(name="xpool", bufs=NCH) as xpool,
            tc.tile_pool(name="ypool", bufs=NCH) as ypool,
            tc.tile_pool(name="dpool", bufs=2) as dpool,
            tc.tile_pool(name="qpool", bufs=2) as qpool,
            tc.tile_pool(name="opool", bufs=1) as opool,
            tc.tile_pool(name="psum", bufs=1, space=bass.MemorySpace.PSUM) as psum,
        ):
            # C = A^T A accumulator, rows 0-127 and 128-191
            cps1 = psum.tile([128, RW], f32, tag="cps1")
            cps2 = psum.tile([F, RW], f32, tag="cps2")
            osb = opool.tile([128, 2 * G + NCH], f32, tag="osb")

            # per-partition contiguous views
            xv = xs[:].rearrange("(p t) d -> p (t d)", p=128)
            yv = ys[:].rearrange("(p t) d -> p (t d)", p=128)

            # pieces of the aw stream: (row0, tpb).  The last block is
            # split in half so the final matmul drain after the DMA
            # stream ends is short.
            pieces = [(b * BLK, TPB) for b in range(NBLK - 1)]
            pieces += [((NBLK - 1) * BLK, TPB // 2),
                       ((NBLK - 1) * BLK + BLK // 2, TPB // 2)]
            n_t = NBLK * TPB
            ti = 0
            for b, (r0, tpb) in enumerate(pieces):
                # A block; tile is padded so the last sub-tile's 256-wide
                # moving operand stays in bounds.  The pad is never
                # initialized: its products only reach PSUM columns
                # 192-255, which are never read back.
                at = apool.tile([128, TPB * G + PAD], f32r)
                eng = nc.sync if (b % 2 == 0) else nc.scalar
                awp = aw[r0:r0 + 128 * tpb, :].rearrange(
                    "(p t) g -> p (t g)", p=128)
                eng.dma_start(at[:, :tpb * G], awp)
                for t in range(tpb):
                    rhs = at[:, t * G:t * G + RW]
                    w1 = at[:, t * G:t * G + 128]
                    w2 = at[:, t * G + 128:t * G + G]
                    nc.tensor.matmul(
                        cps1[:], w1, rhs,
                        start=(ti == 0), stop=(ti == n_t - 1),
                    )
                    nc.tensor.matmul(
                        cps2[:], w2, rhs,
                        start=(ti == 0), stop=(ti == n_t - 1),
                    )
                    ti += 1

                # interleave one MSE chunk every XYB-th A block (last chunk
                # rides after the final A block): the x/y DMA time is the
                # slack that lets the tensor engine keep pace with the aw
                # stream, and the final chunk hides the matmul drain.
                if b % XYB == 1:
                    ch = b // XYB
                    xt = xpool.tile([128, CHW], f32)
                    nc.sync.dma_start(xt[:], xv[:, ch * CHW:(ch + 1) * CHW])
                    yt = ypool.tile([128, CHW], f32)
                    nc.scalar.dma_start(yt[:], yv[:, ch * CHW:(ch + 1) * CHW])
                    dtile = dpool.tile([128, CHW], f32)
                    nc.vector.tensor_sub(dtile[:], xt[:], yt[:])
                    qtile = qpool.tile([128, CHW], f32)
                    nc.scalar.activation(
                        qtile[:], dtile[:],
                        mybir.ActivationFunctionType.Square,
                        accum_out=osb[:, 2 * G + ch:2 * G + ch + 1],
                    )

            # ---- pack C partials next to the MSE partials, single DMA out;
            # k-diagonal extraction happens on host.  The two PSUM reads
            # run on different engines so they don't serialize.
            nc.vector.tensor_copy(osb[:, 0:G], cps1[:, :G])
            nc.vector.tensor_copy(osb[0:F, G:2 * G], cps2[:, :G])
            nc.sync.dma_start(out_d[:], osb[:])

    nc.finalize()
    return nc


def _get_prog() -> bass.Bass:
    global _prog
    if _prog is None:
        _prog = _build()
    return _prog


def _epilogue(C: np.ndarray, sse: float) -> np.ndarray:
    Cr = C.reshape(F, KW, F, KW)
    gram = Cr[:, 0, :, 0] + Cr[:, 1, :, 1] + Cr[:, 2, :, 2]
    norms = np.sqrt(np.diag(gram))
    sim = gram / np.outer(norms, norms)
    mask = (sim > TAU) & (sim <= 1.0) & (~np.eye(F, dtype=bool))
    reg = sim[mask].sum()
    loss = sse / float(B * D) + ALPHA * reg
    return np.asarray(np.float32(loss))


def kernel(x_batch: np.ndarray, y_batch: np.ndarray, conv_w: np.ndarray) -> np.ndarray:
    nc = _get_prog()
    A = np.ascontiguousarray(conv_w.reshape(M, G))
    in_maps = []
    for c in range(N_CORES):
        in_maps.append({
            "xs": np.ascontiguousarray(x_batch[c * ROWS:(c + 1) * ROWS]),
            "ys": np.ascontiguousarray(y_batch[c * ROWS:(c + 1) * ROWS]),
            "aw": np.ascontiguousarray(A[c * MC:(c + 1) * MC]),
        })
    res = run_bass_kernel_spmd(nc, in_maps, core_ids=list(range(N_CORES))).results
    C = np.zeros((G, G), np.float64)
    sse = 0.0
    for r in res:
        o = r["out_d"].astype(np.float64)
        C[0:128] += o[:, 0:G]
        C[128:G] += o[0:F, G:2 * G]
        sse += float(o[:, 2 * G:].sum())
    return _epilogue(C, sse)


# revision 20
# speedup vs baseline: 1.1498x; 1.0082x over previous
"""Trainium2 Bass kernel for nn_EnhanceDiversityFeatureExtracition.

Computes  loss = mean((x-y)^2) + ALPHA * diversity_reg(conv_w)
where diversity_reg builds a 64x64 Gram matrix of the F=64 slices
conv_w[:, :, i, :] (each flattened to a 786432-vector), normalizes it to
cosine similarities, and sums the entries with tau < sim <= 1 off the
diagonal.

Distribution (8 NeuronCores, SPMD):
  - x_batch / y_batch sharded on batch dim: 256 rows per core.
  - conv_w viewed as A = conv_w.reshape(262144, 192)  (row m = (o,c),
    col = f*3+k).  gram[i,j] = sum_m sum_k A[m,3i+k]*A[m,3j+k], so A is
    sharded along the 262144-row reduction axis: 32768 rows per core.
  - Each core returns its partial C = A^T A (192x192) and per-partition
    partial sums of (x-y)^2; the host sums the partials, extracts the
    k-diagonals gram[i,j] = sum_k C[3i+k,3j+k], and applies the tiny
    64x64 masked-similarity epilogue.

On-core dataflow:
  - A shard is streamed in 32 blocks of 1024 rows laid out as
    [128 partitions x 1536 floats] (per-partition contiguous 6KB DMA).
    Each block yields 8*2 = 16 matmuls (contraction 128, strided
    256-wide fp32r moving operands) accumulating into two PSUM tiles
    (C rows 0-127 and 128-191) across the whole shard.
  - DMA issue is split across both HWDGE queues (sync + scalar
    engines) so descriptor generation is not serialized on one queue.
  - MSE: 4 chunks of [128 x 2048] per operand; DVE computes d = x-y,
    ACT computes Square(d) with a per-partition accumulate.
"""

import numpy as np

import concourse.bass as bass
import concourse.mybir as mybir
from concourse import bacc, tile
from concourse.bass_utils import run_bass_kernel_spmd

N_CORES = 8
B, D = 2048, 4096            # x_batch / y_batch
M, G = 262144, 192           # conv_w as (M, G); G = F*KW
F, KW = 64, 3
ROWS = B // N_CORES          # 256 batch rows per core
MC = M // N_CORES            # 32768 reduction rows per core
TPB = 8                      # 128-row tiles per DMA block
BLK = 128 * TPB              # 1024 rows per block
NBLK = MC // BLK             # 32
NCH = 8                      # MSE chunks per core
CHW = (ROWS * D) // (128 * NCH)  # 1024 floats per partition per chunk
XYB = 4                      # one MSE chunk rides every XYB-th A block

ALPHA = 0.0005
TAU = 0.2

_prog = None


def _build() -> bass.Bass:
    nc = bacc.Bacc(None, target_bir_lowering=False)
    f32 = mybir.dt.float32
    f32r = mybir.dt.float32r

    xs = nc.dram_tensor("xs", [ROWS, D], f32, kind="ExternalInput")
    ys = nc.dram_tensor("ys", [ROWS, D], f32, kind="ExternalInput")
    aw = nc.dram_tensor("aw", [MC, G], f32r, kind="ExternalInput")
    # packed output: cols 0:192 = C rows 0-127; cols 192:384 (rows 0-63)
    # = C rows 128-191; cols 384:384+NCH = per-partition MSE partials
    out_d = nc.dram_tensor("out_d", [128, 2 * G + NCH], f32, kind="ExternalOutput")

    # moving operand width for the fp32r full-rate mode (must be >= 256)
    RW = 256
    PAD = RW - G  # 64 junk columns beyond each 192-col tile (never read back)

    with tile.TileContext(nc) as tc:
        with (
            tc.tile_pool(name="apool", bufs=15) as apool,
            tc.tile_pool(name="xpool", bufs=NCH) as xpool,
            tc.tile_pool(name="ypool", bufs=NCH) as ypool,
            tc.tile_pool(name="dpool", bufs=2) as dpool,
            tc.tile_pool(name="qpool", bufs=2) as qpool,
            tc.tile_pool(name="opool", bufs=1) as opool,
            tc.tile_pool(name="psum", bufs=1, space=bass.MemorySpace.PSUM) as psum,
        ):
            # C = A^T A accumulator, rows 0-127 and 128-191
            cps1 = psum.tile([128, RW], f32, tag="cps1")
            cps2 = psum.tile([F, RW], f32, tag="cps2")
            osb = opool.tile([128, 2 * G + NCH], f32, tag="osb")

            # per-partition contiguous views
            xv = xs[:].rearrange("(p t) d -> p (t d)", p=128)
            yv = ys[:].rearrange("(p t) d -> p (t d)", p=128)

            # pieces of the aw stream: (row0, tpb).  The last block
            # tapers into half + quarter + quarter pieces so the final
            # matmul drain after the DMA stream ends is short.
            pieces = [(b * BLK, TPB) for b in range(NBLK - 1)]
            r = (NBLK - 1) * BLK
            pieces += [(r, TPB // 2),
                       (r + BLK // 2, TPB // 4),
                       (r + 3 * BLK // 4, TPB // 4)]
            n_t = NBLK * TPB
            ti = 0
            for b, (r0, tpb) in enumerate(pieces):
                # A block; tile is padded so the last sub-tile's 256-wide
                # moving operand stays in bounds.  The pad is never
                # initialized: its products only reach PSUM columns
                # 192-255, which are never read back.
                at = apool.tile([128, TPB * G + PAD], f32r)
                eng = nc.sync if (b % 2 == 0) else nc.scalar
                awp = aw[r0:r0 + 128 * tpb, :].rearrange(
                    "(p t) g -> p (t g)", p=128)
                eng.dma_start(at[:, :tpb * G], awp)
                for t in range(tpb):
                    rhs = at[:, t * G:t * G + RW]
                    w1 = at[:, t * G:t * G + 128]
                    w2 = at[:, t * G + 128:t * G + G]
                    nc.tensor.matmul(
                        cps1[:], w1, rhs,
                        start=(ti == 0), stop=(ti == n_t - 1),
                    )
                    nc.tensor.matmul(
                        cps2[:], w2, rhs,
                        start=(ti == 0), stop=(ti == n_t - 1),
                    )
                    ti += 1

                # interleave one MSE chunk every XYB-th A block (last chunk
                # rides after the final A block): the x/y DMA time is the
                # slack that lets the tensor engine keep pace with the aw
                # stream, and the final chunk hides the matmul drain.
                # the final chunk rides after the last full block so the
                # tensor engine catches up right before the taper pieces
                if (b % XYB == 1 and b < NBLK - 3) or b == NBLK - 2:
                    ch = NCH - 1 if b == NBLK - 2 else b // XYB
                    xt = xpool.tile([128, CHW], f32)
                    nc.sync.dma_start(xt[:], xv[:, ch * CHW:(ch + 1) * CHW])
                    yt = ypool.tile([128, CHW], f32)
                    nc.scalar.dma_start(yt[:], yv[:, ch * CHW:(ch + 1) * CHW])
                    dtile = dpool.tile([128, CHW], f32)
                    nc.vector.tensor_sub(dtile[:], xt[:], yt[:])
                    qtile = qpool.tile([128, CHW], f32)
                    nc.scalar.activation(
                        qtile[:], dtile[:],
                        mybir.ActivationFunctionType.Square,
                        accum_out=osb[:, 2 * G + ch:2 * G + ch + 1],
                    )

            # ---- pack C partials next to the MSE partials, single DMA out;
            # k-diagonal extraction happens on host.  The two PSUM reads
            # run on different engines so they don't serialize.
            nc.vector.tensor_copy(osb[:, 0:G], cps1[:, :G])
            nc.vector.tensor_copy(osb[0:F, G:2 * G], cps2[:, :G])
            nc.sync.dma_start(out_d[:], osb[:])

    nc.finalize()
    return nc


def _get_prog() -> bass.Bass:
    global _prog
    if _prog is None:
        _prog = _build()
    return _prog


def _epilogue(C: np.ndarray, sse: float) -> np.ndarray:
    Cr = C.reshape(F, KW, F, KW)
    gram = Cr[:, 0, :, 0] + Cr[:, 1, :, 1] + Cr[:, 2, :, 2]
    norms = np.sqrt(np.diag(gram))
    sim = gram / np.outer(norms, norms)
    mask = (sim > TAU) & (sim <= 1.0) & (~np.eye(F, dtype=bool))
    reg = sim[mask].sum()
    loss = sse / float(B * D) + ALPHA * reg
    return np.asarray(np.float32(loss))


def kernel(x_batch: np.ndarray, y_batch: np.ndarray, conv_w: np.ndarray) -> np.ndarray:
    nc = _get_prog()
    A = np.ascontiguousarray(conv_w.reshape(M, G))
    in_maps = []
    for c in range(N_CORES):
        in_maps.append({
            "xs": np.ascontiguousarray(x_batch[c * ROWS:(c + 1) * ROWS]),
            "ys": np.ascontiguousarray(y_batch[c * ROWS:(c + 1) * ROWS]),
            "aw": np.ascontiguousarray(A[c * MC:(c + 1) * MC]),
        })
    res = run_bass_kernel_spmd(nc, in_maps, core_ids=list(range(N_CORES))).results
    C = np.zeros((G, G), np.float64)
    sse = 0.0
    for r in res:
        o = r["out_d"].astype(np.float64)
        C[0:128] += o[:, 0:G]
        C[128:G] += o[0:F, G:2 * G]
        sse += float(o[:, 2 * G:].sum())
    return _epilogue(C, sse)
